# revision 5
# baseline (speedup 1.0000x reference)
"""Cross-attention Trainium2 Bass kernel (nn_CrossAttention, B=4, Sq=Skv=2048,
query_dim=1024, kv_dim=768, H=16, D=64) on 8 NeuronCores.

Sharding: core c -> (batch b = c//2, head-group g = c%2 of 8 heads = 512 dims).
Each core computes its head-group's Q/K/V projections, attention, and a
partial output projection (ctx_g @ Wo_g). Host sums the two partials per
batch and adds the bias terms (bo + bv @ Wo, exact because softmax rows sum
to 1, so the V-bias contributes bv @ Wo to every row).

Device layout:
  - Activations enter transposed (host transposes): qT/kT/vT [dim, seq].
  - Q/K projections produce QT/KT in [head-dim, seq] "pair layout": dd-tile
    t (128 partitions) = heads 2t (partitions 0:64) and 2t+1 (64:128).
  - Scores are computed transposed: S^T[j, q] = KT_h^T @ QT_h (contraction
    over the 64 head dims on partitions), so softmax's j axis lands on
    partitions and E=exp(S^T) is directly the moving operand of the ctx
    matmul with lhsT = V_h (natural [j, d] layout, no transposes).
  - The two heads of a pair run concurrent S matmuls (row groups 0/64) into
    one [128, 1024] two-bank PSUM tile; ONE 1024-wide exp amortizes the
    ~240ns ACT fixed overhead; ctx matmuls trail one jc behind (software
    pipeline) so PE and ACT overlap.
  - V is augmented with a ones column (65th) -> ctx matmul row 64 yields
    the softmax denominators for free.
  - Normalization: sums staged on partitions 64/96, repacked by DMA to
    [8, q], reciprocal on 8 lanes, broadcast back to 64 partitions by
    step-0-free-dim DMA, multiplied into ctx during the f32r cast.
  - All matmul operands are float32r (1 cyc/row at N=512 vs 4 for fp32);
    producers write f32r natively (walrus requires rounded producers).
"""

import sys

sys.path.insert(0, "/opt/trn_rl_repo")

import numpy as np

import concourse.bass as bass  # noqa: F401
import concourse.tile as tile
from concourse import bacc, mybir
from concourse.bass_utils import run_bass_kernel_spmd

F32 = mybir.dt.float32
F32R = mybir.dt.float32r
EXP = mybir.ActivationFunctionType.Exp

QDIM = 1024
KVDIM = 768
H_CORE = 8  # heads per core
D = 64
GDIM = H_CORE * D  # 512, head-group dims per core
KQ = QDIM // 128  # 8  k-chunks for Q proj
KKV = KVDIM // 128  # 6  k-chunks for K/V proj
NB = 512  # q-block size
VCOL = D + 1  # 65, V columns incl. ones


def build_program(sq: int, skv: int):
    """Build the per-core Bass program. Returns nc."""
    nc = bacc.Bacc("TRN2", target_bir_lowering=False, debug=False)

    qt_d = nc.dram_tensor("qT", [KQ, 128, sq], F32R, kind="ExternalInput")
    kt_d = nc.dram_tensor("kT", [KKV, 128, skv], F32R, kind="ExternalInput")
    vt_d = nc.dram_tensor("vT", [KKV, 128, skv], F32R, kind="ExternalInput")
    wq_d = nc.dram_tensor("wq", [KQ, 128, GDIM], F32R, kind="ExternalInput")
    wk_d = nc.dram_tensor("wk", [KKV, 128, GDIM], F32R, kind="ExternalInput")
    wv_d = nc.dram_tensor("wv", [KKV, 128, GDIM], F32R, kind="ExternalInput")
    wo_d = nc.dram_tensor("wo", [4, 128, QDIM], F32R, kind="ExternalInput")
    bq_d = nc.dram_tensor("bq", [4, 128], F32, kind="ExternalInput")
    bk_d = nc.dram_tensor("bk", [4, 128], F32, kind="ExternalInput")
    out_d = nc.dram_tensor("out", [sq, QDIM], F32, kind="ExternalOutput")

    n_qb = sq // NB  # q blocks
    n_jc = skv // 128  # kv chunks (j tiles)
    s_scale = 1.0 / np.sqrt(D)

    with tile.TileContext(nc) as tc:
        with (
            tc.tile_pool(name="sb", bufs=1) as sb,
            tc.tile_pool(name="ps", bufs=1, space="PSUM") as ps,
        ):
            # ---- resident weights (K/V first: they gate the startup) ----
            wk_sb = sb.tile([128, KKV, GDIM], F32R, tag="wk")
            wv_sb = sb.tile([128, KKV, GDIM], F32R, tag="wv")
            for kc in range(KKV):
                nc.sync.dma_start(wk_sb[:, kc, :], wk_d.ap()[kc])
                nc.sync.dma_start(wv_sb[:, kc, :], wv_d.ap()[kc])
            bk_sb = sb.tile([128, 4], F32, tag="bk")
            nc.sync.dma_start(bk_sb, bk_d.ap().rearrange("t p -> p t"))
            ones_f32 = sb.tile([128, 1], F32, tag="ones")
            nc.vector.memset(ones_f32, 1.0)

            # ---- resident K^T (pair layout) and V (+ones) ----
            kt_sb = sb.tile([128, 4, skv], F32R, tag="ktr")
            v_sb = sb.tile([128, n_jc, H_CORE * VCOL], F32R, tag="vsb")
            for jo in range(n_jc):
                nc.vector.tensor_copy(
                    v_sb[:, jo, :].rearrange("p (h d) -> p h d", d=VCOL)[:, :, D : D + 1],
                    ones_f32[:, 0:1].to_broadcast((128, H_CORE, 1)),
                )

            def proj_psums(n):
                """n accumulator psum tiles [128, 512] using st(2-bank)+mm tags."""
                big = ps.tile([128, 1024], F32, tag="st", bufs=2, name="pp_big")
                tiles = [big[:, 0:512], big[:, 512:1024]]
                for i in range(n - 2):
                    t = ps.tile([128, 512], F32, tag="mm", bufs=2, name=f"pp_{i}")
                    tiles.append(t)
                return tiles

            # K and V projections, interleaved per 512-column chunk
            for q4 in range(skv // 512):
                kps = proj_psums(4)
                for kc in range(KKV):
                    ktc = sb.tile([128, 512], F32R, tag="chunk", bufs=2, name="ktc")
                    nc.sync.dma_start(ktc, kt_d.ap()[kc, :, q4 * 512 : (q4 + 1) * 512])
                    for t in range(4):
                        nc.tensor.matmul(
                            kps[t],
                            wk_sb[:, kc, t * 128 : (t + 1) * 128],
                            ktc,
                            start=(kc == 0),
                            stop=(kc == KKV - 1),
                            skip_group_check=True,
                        )
                for t in range(4):
                    nc.vector.tensor_scalar_add(
                        out=kt_sb[:, t, q4 * 512 : (q4 + 1) * 512],
                        in0=kps[t],
                        scalar1=bk_sb[:, t : t + 1],
                    )

                vps = proj_psums(4)
                for kc in range(KKV):
                    vtc = sb.tile([128, 512], F32R, tag="chunk", bufs=2, name="vtc")
                    nc.sync.dma_start(vtc, vt_d.ap()[kc, :, q4 * 512 : (q4 + 1) * 512])
                    for t in range(4):
                        nc.tensor.matmul(
                            vps[t],
                            vtc[:, t * 128 : (t + 1) * 128],
                            wv_sb[:, kc, :],
                            start=(kc == 0),
                            stop=(kc == KKV - 1),
                            skip_group_check=True,
                        )
                for t in range(4):
                    jo = q4 * 4 + t
                    nc.vector.tensor_copy(
                        v_sb[:, jo, :].rearrange("p (h d) -> p h d", d=VCOL)[
                            :, :, 0:D
                        ],
                        vps[t].rearrange("p (h d) -> p h d", d=D),
                    )

            # Q/O weights arrive after the K/V projections are underway
            wq_sb = sb.tile([128, KQ, GDIM], F32R, tag="wq")
            nc.sync.dma_start(wq_sb, wq_d.ap().rearrange("k p n -> p k n"))
            wo_sb = sb.tile([128, 4, QDIM], F32R, tag="wo")
            nc.sync.dma_start(wo_sb, wo_d.ap().rearrange("k p n -> p k n"))
            bq_sb = sb.tile([128, 4], F32, tag="bq")
            nc.sync.dma_start(bq_sb, bq_d.ap().rearrange("t p -> p t"))

            def emit_out_proj(ctxn_t, qb_i):
                # out projection: out[s, n] = ctxn^T @ Wo_g  (partial)
                for sti in range(NB // 128):
                    osb = sb.tile([128, QDIM], F32, tag="osb", bufs=2, name="osb")
                    for nh in range(2):
                        ops = ps.tile([128, 512], F32, tag="mm", bufs=2, name="ops")
                        for c in range(4):
                            nc.tensor.matmul(
                                ops,
                                ctxn_t[:, c, sti * 128 : (sti + 1) * 128],
                                wo_sb[:, c, nh * 512 : (nh + 1) * 512],
                                start=(c == 0),
                                stop=(c == 3),
                                skip_group_check=True,
                            )
                        nc.vector.tensor_copy(osb[:, nh * 512 : (nh + 1) * 512], ops)
                    r0 = qb_i * NB + sti * 128
                    nc.sync.dma_start(out_d.ap()[r0 : r0 + 128, :], osb)

            prev_ctxn = None
            prev_qb = -1

            # ---- per q-block: Q proj, attention (out proj trails 1 block) ----
            for qb in range(n_qb):
                qsl = slice(qb * NB, (qb + 1) * NB)

                # Q projection, 2 dd-tiles at a time (mm tag only, 2 banks)
                qt_blk = sb.tile([128, 4, NB], F32R, tag="qt", bufs=2, name="qt_blk")
                for half in range(2):
                    qps = [
                        ps.tile([128, 512], F32, tag="mm", bufs=2, name=f"qps{t}")
                        for t in range(2)
                    ]
                    for kc in range(KQ):
                        qtc = sb.tile([128, NB], F32R, tag="qchunk", bufs=4, name="qtc")
                        nc.sync.dma_start(qtc, qt_d.ap()[kc, :, qsl])
                        for t in range(2):
                            dd = half * 2 + t
                            nc.tensor.matmul(
                                qps[t],
                                wq_sb[:, kc, dd * 128 : (dd + 1) * 128],
                                qtc,
                                start=(kc == 0),
                                stop=(kc == KQ - 1),
                                skip_group_check=True,
                            )
                    for t in range(2):
                        dd = half * 2 + t
                        nc.vector.tensor_scalar_add(
                            out=qt_blk[:, dd, :],
                            in0=qps[t],
                            scalar1=bq_sb[:, dd : dd + 1],
                        )

                if prev_ctxn is not None:
                    emit_out_proj(prev_ctxn, prev_qb)

                # attention: pairs of heads, 1024-wide exp, SW-pipelined ctx
                ctxn = sb.tile([128, 4, NB], F32R, tag="ctxn", bufs=2, name="ctxn")
                for pair in range(4):
                    hA, hB = 2 * pair, 2 * pair + 1
                    ctx_a = ps.tile([128, NB], F32, tag="ctx", bufs=2, name="ctx_a")
                    ctx_b = ps.tile([128, NB], F32, tag="ctx", bufs=2, name="ctx_b")
                    e_prev = None
                    for jc in range(n_jc):
                        st_ps = ps.tile(
                            [128, 2 * NB], F32, tag="st", bufs=2, name="st_ps"
                        )
                        jsl = slice(jc * 128, (jc + 1) * 128)
                        nc.tensor.matmul(
                            st_ps[:, 0:NB],
                            kt_sb[0:64, pair, jsl],
                            qt_blk[0:64, pair, :],
                            start=True,
                            stop=True,
                            skip_group_check=True,
                        )
                        nc.tensor.matmul(
                            st_ps[:, NB : 2 * NB],
                            kt_sb[64:128, pair, jsl],
                            qt_blk[64:128, pair, :],
                            start=True,
                            stop=True,
                            skip_group_check=True,
                        )
                        e_t = sb.tile([128, 2 * NB], F32R, tag="e", bufs=2, name="e_t")
                        nc.scalar.activation(out=e_t, in_=st_ps, func=EXP, scale=s_scale)
                        if e_prev is not None:
                            pj = jc - 1
                            nc.tensor.matmul(
                                ctx_a[0:VCOL, :],
                                v_sb[:, pj, hA * VCOL : (hA + 1) * VCOL],
                                e_prev[:, 0:NB],
                                start=(pj == 0),
                                stop=False,
                                skip_group_check=True,
                            )
                            nc.tensor.matmul(
                                ctx_b[0:VCOL, :],
                                v_sb[:, pj, hB * VCOL : (hB + 1) * VCOL],
                                e_prev[:, NB : 2 * NB],
                                start=(pj == 0),
                                stop=False,
                                skip_group_check=True,
                            )
                        e_prev = e_t
                    pj = n_jc - 1
                    nc.tensor.matmul(
                        ctx_a[0:VCOL, :],
                        v_sb[:, pj, hA * VCOL : (hA + 1) * VCOL],
                        e_prev[:, 0:NB],
                        start=False,
                        stop=True,
                        skip_group_check=True,
                    )
                    nc.tensor.matmul(
                        ctx_b[0:VCOL, :],
                        v_sb[:, pj, hB * VCOL : (hB + 1) * VCOL],
                        e_prev[:, NB : 2 * NB],
                        start=False,
                        stop=True,
                        skip_group_check=True,
                    )
                    # per-pair normalization (overlaps next pair's attention):
                    # sums at psum row 64 -> stage partitions 64/96 -> DMA to
                    # [2, NB] -> reciprocal -> broadcast -> multiply
                    stage = sb.tile([128, NB], F32, tag="stage", bufs=1, name="stage")
                    nc.vector.tensor_copy(stage[64:65, :], ctx_a[64:65, :])
                    nc.vector.tensor_copy(stage[96:97, :], ctx_b[64:65, :])
                    ctxu = sb.tile([128, NB], F32, tag="ctxu", bufs=2, name="ctxu")
                    nc.vector.tensor_copy(ctxu[0:64, :], ctx_a[0:64, :])
                    nc.vector.tensor_copy(ctxu[64:128, :], ctx_b[0:64, :])
                    sums_p = sb.tile([2, NB], F32, tag="sums", bufs=1, name="sums_p")
                    nc.sync.dma_start(sums_p[0:1, :], stage[64:65, :])
                    nc.sync.dma_start(sums_p[1:2, :], stage[96:97, :])
                    rsum_p = sb.tile([2, NB], F32, tag="rsum", bufs=1, name="rsum_p")
                    nc.vector.reciprocal(out=rsum_p, in_=sums_p)
                    rb = sb.tile([128, NB], F32, tag="rb", bufs=1, name="rb")
                    for sub in range(2):
                        nc.sync.dma_start(
                            rb[sub * 64 : sub * 64 + 64, :],
                            rsum_p[sub : sub + 1, None, :].to_broadcast((1, 64, NB)),
                        )
                    nc.vector.tensor_mul(
                        out=ctxn[:, pair, :], in0=ctxu, in1=rb
                    )

                prev_ctxn = ctxn
                prev_qb = qb

            # final block's out projection
            emit_out_proj(prev_ctxn, prev_qb)

    nc.compile()
    return nc


_NC_CACHE = {}


def _get_nc(sq, skv):
    key = (sq, skv)
    if key not in _NC_CACHE:
        _NC_CACHE[key] = build_program(sq, skv)
    return _NC_CACHE[key]


def make_in_maps(query, key, value, Wq, bq, Wk, bk, Wv, bv, Wo, bo):
    B = query.shape[0]
    f = np.float32
    per_batch = []
    for b in range(B):
        per_batch.append(
            (
                np.ascontiguousarray(query[b].T, f).reshape(KQ, 128, -1),
                np.ascontiguousarray(key[b].T, f).reshape(KKV, 128, -1),
                np.ascontiguousarray(value[b].T, f).reshape(KKV, 128, -1),
            )
        )
    per_group = []
    for g in range(2):
        gs = slice(g * GDIM, (g + 1) * GDIM)
        per_group.append(
            dict(
                wq=np.ascontiguousarray(Wq[:, gs], f).reshape(KQ, 128, GDIM),
                wk=np.ascontiguousarray(Wk[:, gs], f).reshape(KKV, 128, GDIM),
                wv=np.ascontiguousarray(Wv[:, gs], f).reshape(KKV, 128, GDIM),
                wo=np.ascontiguousarray(Wo[gs, :], f).reshape(4, 128, QDIM),
                bq=np.ascontiguousarray(bq[gs], f).reshape(4, 128),
                bk=np.ascontiguousarray(bk[gs], f).reshape(4, 128),
            )
        )
    in_maps = []
    for c in range(2 * B):
        b, g = c // 2, c % 2
        qT, kT, vT = per_batch[b]
        m = dict(qT=qT, kT=kT, vT=vT)
        m.update(per_group[g])
        in_maps.append(m)
    return in_maps


def kernel(query, key, value, Wq, bq, Wk, bk, Wv, bv, Wo, bo, _trace=False):
    B, sq, _ = query.shape
    skv = key.shape[1]
    nc = _get_nc(sq, skv)
    in_maps = make_in_maps(query, key, value, Wq, bq, Wk, bk, Wv, bv, Wo, bo)
    res = run_bass_kernel_spmd(
        nc, in_maps, core_ids=list(range(len(in_maps))), trace=_trace
    )
    bias_eff = (
        bo.astype(np.float64) + bv.astype(np.float64) @ Wo.astype(np.float64)
    ).astype(np.float32)
    out = np.empty((B, sq, QDIM), np.float32)
    for b in range(B):
        out[b] = res.results[2 * b]["out"] + res.results[2 * b + 1]["out"] + bias_eff
    if _trace:
        return out, res
    return out
